# revision 82
# baseline (speedup 1.0000x reference)
"""Trainium2 Bass kernel for nn_Block_55336358643145 (dense transformer block).

Head-sharded attention, row-sharded MLP, one SPMD program for 8 cores:

- Every core runs LN1 over ALL 4096 rows (replicated; DVE stats + ACT affine,
  row-major -> DRAM bounce -> xbar DMA-transpose into xT, no PE transposes).
- QKV tensor-parallel: core c computes q/k/v only for its 2 heads (its 128-col
  slice of Wq/Wk/Wv arrives as per-core input data; the program is identical).
  q is written directly into checkerboard-zeroed qz tiles so one scores matmul
  [128x128x512] serves both heads; k lands transposed [dims, kpos]; v is
  xbar-transposed to row-major with a per-head ones column (softmax denominator
  via the attn@v matmul). The every-25th-kpos column mask is applied by zeroing
  v_aug rows (numerator AND denominator), so exp needs no bias at all.
- Attention per core: 2 heads x full causal structure -> identical shape on
  every core, no padding waste, static masks.
- A tiny 1MB AllToAll exchanges normalized per-head outputs so each core
  assembles the full 1024-dim attention output for its own 512 rows; the
  8-chunk contraction of the out projection then runs locally in PSUM.
- Out-proj + residual, LN2, 4x MLP with exact-erf Gelu, residual, store:
  row-sharded exactly like the classic data-parallel layout.
"""

import contextlib

import numpy as np

import concourse.bass as bass
import concourse.tile as tile
from concourse import bacc, mybir
from concourse.bass_utils import run_bass_kernel_spmd

F32 = mybir.dt.float32
BF16 = mybir.dt.bfloat16
AF = mybir.ActivationFunctionType
ALU = mybir.AluOpType

B, S, D, H, HD, FF = 2, 2048, 1024, 16, 64, 4096
NCORE = 8
NROW = B * S          # 4096 rows total
R = NROW // NCORE     # 512 rows per core (MLP phase)
NG = NROW // 512      # 8 rowgroups of 512 rows
DC = D // 128         # 8 d-chunks
GC = FF // 128        # 32 mlp hidden chunks
NT = S // 128         # 16 kpos tiles per batch
NJ = S // 256         # 8 q-tiles of 256 per batch
LN_EPS = 1e-5
JD = 25               # joined dim for the column-zero mask
NEG = -1.0e30


def build_program(apply_ln1, apply_ln2, dbg=False):
    nc = bacc.Bacc("TRN2", target_bir_lowering=False, debug=False,
                   num_devices=NCORE)

    def inp(name, shape, dt=F32):
        return nc.dram_tensor(name, list(shape), dt, kind="ExternalInput").ap()

    io = dict(
        hs=inp("hs", (NROW, D)),
        hso=inp("hso", (R, D)),
        wq=inp("wq", (D, 128), BF16), wk=inp("wk", (D, 128), BF16),
        wv=inp("wv", (D, 128), BF16),
        wp=inp("wp", (D, D), BF16),
        w1=inp("w1", (GC, 128, DC, 128), BF16), w2=inp("w2", (FF, D), BF16),
        bq8=inp("bq8", (128, 1)), bk1=inp("bk1", (128, 1)),
        bv1=inp("bv1", (128, 1)),
        b1l=inp("b1l", (128, GC)),
        bpr=inp("bpr", (1, D), BF16), b2r=inp("b2r", (1, D), BF16),
        onesr=inp("onesr", (1, 128), BF16),
        ln1gb=inp("ln1gb", (2, D)), ln2gb=inp("ln2gb", (2, D)),
        maskd=inp("maskd", (128, 2, 2, 256)),
        colz=inp("colz", (128, NT)),
        vmask=inp("vmask", (128, 4, 512), BF16),
        colz2=inp("colz2", (128, NT, 2), BF16),
        trim=inp("trim", (128, 2, 2, 256), BF16),
        ident=inp("ident", (128, 128), BF16),
        out=nc.dram_tensor("out", [R, D], F32, kind="ExternalOutput").ap(),
    )
    if dbg:
        io["xdump"] = nc.dram_tensor("xdump", [NROW, D], BF16,
                                     kind="ExternalOutput").ap()
        io["odump"] = nc.dram_tensor("odump", [NCORE, 128, R], BF16,
                                     kind="ExternalOutput").ap()
        io["hdump"] = nc.dram_tensor("hdump", [R, D], BF16,
                                     kind="ExternalOutput").ap()
        io["kdump"] = nc.dram_tensor("kdump", [NG, 128, 512], BF16,
                                     kind="ExternalOutput").ap()
        io["qdump"] = nc.dram_tensor("qdump", [NG, 128, 2, 512], BF16,
                                     kind="ExternalOutput").ap()
        io["vdump"] = nc.dram_tensor("vdump", [NG, 128, 4, 2, 80], BF16,
                                     kind="ExternalOutput").ap()

    with tile.TileContext(nc) as tc:
        _build(tc, io, apply_ln1, apply_ln2)
    nc.compile()
    return nc


def _build(tc, io, apply_ln1, apply_ln2):
    nc = tc.nc

    with contextlib.ExitStack() as ctx:
        persist = ctx.enter_context(
            tc.tile_pool(name="persist", bufs=1, side="left"))
        dram = ctx.enter_context(tc.tile_pool(name="dram", bufs=1,
                                              space="DRAM"))

        # ---- constants + weights ------------------------------------------
        eps_sb = persist.tile([128, 1], F32)
        nc.vector.memset(eps_sb[:], LN_EPS)
        ones_r = persist.tile([1, 128], BF16)
        nc.sync.dma_start(ones_r[:], io["onesr"][:])
        bq8_sb = persist.tile([128, 1], F32)
        nc.sync.dma_start(bq8_sb[:], io["bq8"][:])
        bk1_sb = persist.tile([128, 1], F32)
        nc.sync.dma_start(bk1_sb[:], io["bk1"][:])
        bv1_sb = persist.tile([128, 1], F32)
        nc.sync.dma_start(bv1_sb[:], io["bv1"][:])
        b1l_sb = persist.tile([128, GC], F32)
        nc.sync.dma_start(b1l_sb[:], io["b1l"][:])
        bpr_sb = persist.tile([1, D], BF16)
        nc.sync.dma_start(bpr_sb[:], io["bpr"][:])
        b2r_sb = persist.tile([1, D], BF16)
        nc.sync.dma_start(b2r_sb[:], io["b2r"][:])
        maskd_sb = persist.tile([128, 2, 2, 256], F32)
        nc.sync.dma_start(maskd_sb[:], io["maskd"][:])
        colz_sb = persist.tile([128, NT], F32)
        nc.sync.dma_start(colz_sb[:], io["colz"][:])
        vmask_sb = persist.tile([128, 4, 512], BF16)
        nc.sync.dma_start(vmask_sb[:], io["vmask"][:])
        colz2_sb = persist.tile([128, NT, 2], BF16)
        nc.sync.dma_start(colz2_sb[:], io["colz2"][:])
        trim_sb = persist.tile([128, 2, 2, 256], BF16)
        nc.sync.dma_start(trim_sb[:], io["trim"][:])
        ident_sb = persist.tile([128, 128], BF16)
        nc.sync.dma_start(ident_sb[:], io["ident"][:])

        wq_sb = persist.tile([128, DC, 128], BF16)
        nc.sync.dma_start(wq_sb[:],
                          io["wq"].rearrange("(c p) o -> p c o", p=128))
        wk_sb = persist.tile([128, DC, 128], BF16)
        nc.sync.dma_start(wk_sb[:],
                          io["wk"].rearrange("(c p) o -> p c o", p=128))
        wv_sb = persist.tile([128, DC, 128], BF16)
        nc.sync.dma_start(wv_sb[:],
                          io["wv"].rearrange("(c p) o -> p c o", p=128))
        wp_sb = persist.tile([128, DC, D], BF16)
        nc.scalar.dma_start(wp_sb[:],
                            io["wp"].rearrange("(c p) o -> p c o", p=128))

        def ln_gb_tiles(gb_inp, nm):
            g_sb = persist.tile([128, D], F32, name=f"g_{nm}")
            b_sb = persist.tile([128, D], F32, name=f"b_{nm}")
            g_row = persist.tile([1, D], F32, name=f"gr_{nm}")
            b_row = persist.tile([1, D], F32, name=f"br_{nm}")
            nc.sync.dma_start(g_row[:], gb_inp[0:1, :])
            nc.sync.dma_start(b_row[:], gb_inp[1:2, :])
            nc.gpsimd.partition_broadcast(g_sb[:], g_row[:])
            nc.gpsimd.partition_broadcast(b_sb[:], b_row[:])
            return g_sb, b_sb

        ln1_g = ln1_b = ln2_g = ln2_b = None
        if apply_ln1:
            ln1_g, ln1_b = ln_gb_tiles(io["ln1gb"], "ln1")
        if apply_ln2:
            ln2_g, ln2_b = ln_gb_tiles(io["ln2gb"], "ln2")

        def layernorm(dst, src, pool, g_sb, b_sb):
            """dst (any dtype) = LN(src); DVE stats, ACT affine."""
            stats = pool.tile([128, 2, 6], F32, tag="ln_stats")
            sg = src.rearrange("p (g d) -> p g d", g=2)
            for g in range(2):
                nc.vector.bn_stats(out=stats[:, g, :], in_=sg[:, g, :])
            mv = pool.tile([128, 2], F32, tag="ln_mv")
            nc.vector.bn_aggr(out=mv[:], in_=stats[:])
            rstd = pool.tile([128, 1], F32, tag="ln_rstd")
            nc.scalar.activation(out=rstd[:], in_=mv[:, 1:2], func=AF.Sqrt,
                                 bias=eps_sb[:], scale=1.0)
            nc.vector.reciprocal(out=rstd[:], in_=rstd[:])
            nmr = pool.tile([128, 1], F32, tag="ln_nmr")
            nc.vector.tensor_scalar(out=nmr[:], in0=mv[:, 0:1],
                                    scalar1=rstd[:], scalar2=-1.0,
                                    op0=ALU.mult, op1=ALU.mult)
            nc.scalar.activation(out=dst, in_=src, func=AF.Identity,
                                 bias=nmr[:], scale=rstd[:])
            if g_sb is not None:
                nc.vector.tensor_mul(dst, dst, g_sb[:])
                nc.vector.tensor_add(dst, dst, b_sb[:])

        # ---- attention-persistent tiles -----------------------------------
        es_att = ctx.enter_context(contextlib.ExitStack())
        attp = es_att.enter_context(
            tc.tile_pool(name="attp", bufs=1, side="right"))
        kT_t = [attp.tile([128, 4, 128], BF16, name=f"kT_{g}")
                for g in range(NG)]
        va_t = [attp.tile([128, 4, 2, 80], BF16, name=f"va_{g}")
                for g in range(NG)]
        qz_t = [attp.tile([128, 2, 512], BF16, name=f"qz_{g}")
                for g in range(NG)]
        o_sb = attp.tile([128, 2 * NJ, 256], BF16, name="o_sb")

        for g in range(NG):
            nc.gpsimd.memset(qz_t[g][:], 0.0)
            nc.gpsimd.memset(va_t[g][:], 1.0)

        xln_d = dram.tile([NROW, D], BF16)
        o_sendA = dram.tile([NCORE, 128, 256], BF16)
        o_recvA = dram.tile([NCORE, 128, 256], BF16)
        o_sendB = dram.tile([NCORE, 128, 256], BF16)
        o_recvB = dram.tile([NCORE, 128, 256], BF16)

        # ================= P0+P1: LN1 -> xT -> QKV ==========================
        # Two batch-halves: LN tiles stream to DRAM, 8 big xbar transposes
        # per half, then QKV matmuls per rowgroup. Attention on batch 0
        # overlaps the batch-1 half; the PSUM pools are shared between the
        # QKV and attention pipelines so both can be in flight at once.
        with tc.tile_pool(name="p0h", bufs=4, side="left") as p0h, \
             tc.tile_pool(name="p0x", bufs=2, side="left") as p0x, \
             tc.tile_pool(name="p0s", bufs=8, side="left") as p0s, \
             tc.tile_pool(name="xT_p", bufs=2, side="left") as xtp, \
             tc.tile_pool(name="vt_p", bufs=2, side="left") as vtp, \
             tc.tile_pool(name="vt_ps", bufs=2, space="PSUM") as vtps, \
             tc.tile_pool(name="p0ps", bufs=2, space="PSUM") as p0ps:
            for bb in range(2):
                for g in range(4 * bb, 4 * bb + 4):
                    xln = p0x.tile([128, 4, D], BF16, tag="xln")
                    for rt in range(4):
                        row0 = 512 * g + 128 * rt
                        hst = p0h.tile([128, D], F32, tag="hst")
                        nc.sync.dma_start(hst[:],
                                          io["hs"][row0:row0 + 128, :])
                        layernorm(xln[:, rt, :], hst[:], p0s, ln1_g, ln1_b)
                    nc.scalar.dma_start(
                        xln_d[512 * g:512 * (g + 1), :]
                        .rearrange("(a p) d -> p a d", p=128),
                        xln[:])
                xTb = xtp.tile([128, DC, 2048], BF16, tag="xT")
                for c in range(DC):
                    nc.sync.dma_start_transpose(
                        xTb[:, c, :],
                        xln_d[2048 * bb:2048 * (bb + 1),
                              128 * c:128 * (c + 1)])
                for g in range(4 * bb, 4 * bb + 4):
                    xs = slice(512 * (g % 4), 512 * (g % 4) + 512)

                    # q -> checkerboard qz (scaled 1/8)
                    ps = p0ps.tile([128, 512], F32, tag="psq", name=f"psq_{g}")
                    for c in range(DC):
                        nc.tensor.matmul(ps[:], wq_sb[:, c, :], xTb[:, c, xs],
                                         start=(c == 0), stop=(c == DC - 1))
                    for jl in range(2):
                        nc.vector.tensor_scalar(
                            out=qz_t[g][0:64, jl, 0:256],
                            in0=ps[0:64, 256 * jl:256 * (jl + 1)],
                            scalar1=0.125, scalar2=bq8_sb[0:64, :],
                            op0=ALU.mult, op1=ALU.add)
                        nc.vector.tensor_scalar(
                            out=qz_t[g][64:128, jl, 256:512],
                            in0=ps[64:128, 256 * jl:256 * (jl + 1)],
                            scalar1=0.125, scalar2=bq8_sb[64:128, :],
                            op0=ALU.mult, op1=ALU.add)

                    # k -> kT
                    ps = p0ps.tile([128, 512], F32, tag="psk", name=f"psk_{g}")
                    for c in range(DC):
                        nc.tensor.matmul(ps[:], wk_sb[:, c, :], xTb[:, c, xs],
                                         start=(c == 0), stop=(c == DC - 1))
                    nc.scalar.activation(
                        kT_t[g][:].rearrange("p a b -> p (a b)"), ps[:],
                        func=AF.Identity, bias=bk1_sb[:], scale=1.0)

                    # v -> vT -> PE transpose -> v_aug; column-zero folded in
                    ps = p0ps.tile([128, 512], F32, tag="psv", name=f"psv_{g}")
                    for c in range(DC):
                        nc.tensor.matmul(ps[:], wv_sb[:, c, :], xTb[:, c, xs],
                                         start=(c == 0), stop=(c == DC - 1))
                    vT = vtp.tile([128, 512], BF16, tag="vT")
                    nc.scalar.activation(vT[:], ps[:], func=AF.Identity,
                                         bias=bv1_sb[:], scale=1.0)
                    nc.vector.tensor_mul(vT[:], vT[:], vmask_sb[:, g % 4, :])
                    vsc = vtps.tile([128, 4, 128], BF16, tag="vtp",
                                    name=f"vsc_{g}")
                    for t4 in range(4):
                        nc.tensor.transpose(
                            vsc[:, t4, :],
                            vT[:, 128 * t4:128 * (t4 + 1)],
                            ident_sb[:])
                    nc.vector.tensor_copy(
                        va_t[g][:, :, :, 0:64],
                        vsc[:].rearrange("p t (j d) -> p t j d", j=2))
                    for t4 in range(4):
                        tt = 4 * (g % 4) + t4  # kpos tile within batch
                        # ones (denominator) column gets the same kpos mask
                        nc.vector.tensor_copy(
                            va_t[g][:, t4, :, 64:65]
                            .rearrange("p a b -> p (a b)"),
                            colz2_sb[:, tt, :])

        def k_ap(b, t):
            return kT_t[4 * b + t // 4][:, t % 4, :]

        def v_ap(b, t, j):
            return va_t[4 * b + t // 4][:, t % 4, j, 0:65]

        def qz_ap(b, jq):
            return qz_t[4 * b + jq // 2][:, jq % 2, :]

        if "kdump" in io:
            for g in range(NG):
                nc.sync.dma_start(
                    io["kdump"][g],
                    kT_t[g][:].rearrange("p a b -> p (a b)"))
                nc.sync.dma_start(io["qdump"][g], qz_t[g][:])
                nc.sync.dma_start(io["vdump"][g], va_t[g][:])

        # ================= P2: attention ====================================
        es_h = ctx.enter_context(contextlib.ExitStack())
        h_pool = es_h.enter_context(
            tc.tile_pool(name="h_p", bufs=1, side="left"))
        h_sb = h_pool.tile([128, 4, D], F32)
        with tc.tile_pool(name="sc_ps", bufs=2, space="PSUM") as scps, \
             tc.tile_pool(name="oT_ps", bufs=4, space="PSUM") as otps, \
             tc.tile_pool(name="ex_p", bufs=4, side="left") as asb, \
             tc.tile_pool(name="nrm_p", bufs=4, side="left") as anorm:
            order = ([(0, jq) for jq in range(NJ)]
                     + [(1, jq) for jq in range(0, NJ, 2)]
                     + [(1, jq) for jq in range(1, NJ, 2)])
            a2a1_at = len(order) - NJ // 2 - 1  # after the last b1-even
            for idx, (b, jq) in enumerate(order):
                if True:
                    m = NJ * b + jq
                    qz = qz_ap(b, jq)
                    oTs = [otps.tile([65, 256], F32, tag="oT",
                                     name=f"oT_{m}_{j}") for j in range(2)]
                    for u in range(jq + 1):
                        sc = scps.tile([128, 2, 512], F32, tag="sc",
                                       name=f"sc_{m}_{u}")
                        for s in range(2):
                            nc.tensor.matmul(sc[:, s, :], k_ap(b, 2 * u + s),
                                             qz, start=True, stop=True)
                        ex = asb.tile([128, 2, 512], BF16, tag="ex",
                                      name=f"ex_{m}_{u}")
                        nc.scalar.activation(ex[:], sc[:], func=AF.Exp)
                        if u == jq:
                            nc.vector.tensor_mul(
                                ex[:].rearrange("p a b -> p (a b)"),
                                ex[:].rearrange("p a b -> p (a b)"),
                                trim_sb[:].rearrange("p a c b -> p (a c b)"))
                        for j in range(2):
                            for s in range(2):
                                nc.tensor.matmul(
                                    oTs[j][:], v_ap(b, 2 * u + s, j),
                                    ex[:, s, 256 * j:256 * (j + 1)],
                                    start=(u == 0 and s == 0),
                                    stop=(u == jq and s == 1))
                    for j in range(2):
                        den = anorm.tile([1, 256], F32, tag="den",
                                         name=f"den_{m}_{j}")
                        nc.vector.tensor_copy(den[:], oTs[j][64:65, :])
                        rb = anorm.tile([64, 256], F32, tag="rb",
                                        name=f"rb_{m}_{j}")
                        nc.gpsimd.partition_broadcast(rb[:], den[:])
                        nc.vector.reciprocal_approx_fast(out=rb[:], in_=rb[:])
                        if j == 0:
                            nc.vector.tensor_mul(o_sb[0:64, m, :],
                                                 oTs[j][0:64, :], rb[:])
                        else:
                            tmp1 = anorm.tile([64, 256], BF16, tag="tmp1",
                                              name=f"tmp1_{m}")
                            nc.vector.tensor_mul(tmp1[:], oTs[j][0:64, :],
                                                 rb[:])
                            nc.sync.dma_start(o_sb[64:128, m, :], tmp1[:])
                if m % 2 == 1:
                    # odd chunk complete -> ship its A2A-B slice now
                    nc.sync.dma_start(o_sendB[m // 2], o_sb[:, m, :])
                if idx == a2a1_at:
                    # all jq-even outputs done -> ship first A2A half
                    for d in range(NCORE):
                        nc.sync.dma_start(o_sendA[d], o_sb[:, 2 * d, :])
                    nc.gpsimd.collective_compute(
                        "AllToAll", ALU.bypass,
                        replica_groups=[[0, 1, 2, 3, 4, 5, 6, 7]],
                        ins=[o_sendA.opt()], outs=[o_recvA.opt()])
            nc.gpsimd.collective_compute(
                "AllToAll", ALU.bypass,
                replica_groups=[[0, 1, 2, 3, 4, 5, 6, 7]],
                ins=[o_sendB.opt()], outs=[o_recvB.opt()])

            # ======= P4: out-proj + residual (inside attention PSUM scope,
            # borrowing sc-tag banks so rt0/1 overlap the second A2A) =======
            with tc.tile_pool(name="oc_p", bufs=1, side="left") as ocp, \
                 tc.tile_pool(name="hs2", bufs=2, side="left") as hs2:
                oc_sbs = []
                for half, o_recv_h in ((0, o_recvA), (1, o_recvB)):
                    oc_sb = ocp.tile([128, NCORE, 256], BF16,
                                     name=f"oc_{half}")
                    nc.sync.dma_start(oc_sb[:],
                                      o_recv_h[:].rearrange("r p c -> p r c"))
                    oc_sbs.append(oc_sb)
                for rt in range(4):
                    half, rh = rt // 2, rt % 2
                    oc_sb = oc_sbs[half]
                    hst = hs2.tile([128, D], F32, tag="hst", name=f"hso_{rt}")
                    nc.sync.dma_start(hst[:],
                                      io["hso"][128 * rt:128 * (rt + 1), :])
                    for cg in range(2):
                        psw = scps.tile([128, 2, 512], F32, tag="sc",
                                        name=f"ps_wp_{rt}_{cg}")
                        ps = psw[:, 0, :]
                        for r in range(NCORE):
                            nc.tensor.matmul(
                                ps, oc_sb[:, r, 128 * rh:128 * (rh + 1)],
                                wp_sb[:, r, 512 * cg:512 * (cg + 1)],
                                start=(r == 0), stop=False)
                        nc.tensor.matmul(ps, ones_r[:],
                                         bpr_sb[:, 512 * cg:512 * (cg + 1)],
                                         start=False, stop=True)
                        nc.vector.tensor_add(
                            h_sb[:, rt, 512 * cg:512 * (cg + 1)],
                            ps, hst[:, 512 * cg:512 * (cg + 1)])
        es_att.close()  # attention tiles done

        # ================= P5: LN2 + transpose ==============================
        es_mlp = ctx.enter_context(contextlib.ExitStack())
        mlp_pool = es_mlp.enter_context(
            tc.tile_pool(name="mlp_p", bufs=1, side="left"))
        h2T = mlp_pool.tile([128, DC, R], BF16)
        gT = mlp_pool.tile([128, GC, R], BF16)
        h2_d = dram.tile([R, D], BF16)
        with tc.tile_pool(name="p5", bufs=2, side="left") as p5:
            for rt in range(4):
                h2 = p5.tile([128, D], BF16, tag="h2")
                layernorm(h2[:], h_sb[:, rt, :], p5, ln2_g, ln2_b)
                nc.sync.dma_start(h2_d[128 * rt:128 * (rt + 1), :], h2[:])
            for half in range(2):
                rs = slice(256 * half, 256 * (half + 1))
                for c in range(DC):
                    nc.sync.dma_start_transpose(
                        h2T[:, c, rs], h2_d[rs, 128 * c:128 * (c + 1)])

        # ================= P6: MLP up + gelu ================================
        with tc.tile_pool(name="w_w1", bufs=3, side="left") as wpl, \
             tc.tile_pool(name="ps_w1", bufs=2, space="PSUM") as pps:
            for gc in range(GC):
                wt = wpl.tile([128, DC, 128], BF16, tag="w1")
                nc.sync.dma_start(wt[:], io["w1"][gc])
                ps = pps.tile([128, R], F32, tag="ps", name=f"ps_w1_{gc}")
                for c in range(DC):
                    nc.tensor.matmul(ps[:], wt[:, c, :], h2T[:, c, :],
                                     start=(c == 0), stop=(c == DC - 1))
                nc.scalar.activation(gT[:, gc, :], ps[:], func=AF.Gelu,
                                     bias=b1l_sb[:, gc:gc + 1], scale=1.0)

        # ================= P7: MLP down + bias + residual ===================
        with tc.tile_pool(name="w_w2", bufs=3, side="left") as wpl, \
             tc.tile_pool(name="o_sbp", bufs=2, side="left") as osb, \
             tc.tile_pool(name="o_ps", bufs=1, space="PSUM") as pps:
            psts = [pps.tile([128, 512], F32, tag=f"o{i}", name=f"o_ps_{i}")
                    for i in range(8)]
            for gc in range(GC):
                wt = wpl.tile([128, D], BF16, tag="w2")
                nc.sync.dma_start(
                    wt[:], io["w2"][128 * gc:128 * (gc + 1), :])
                for qt in range(4):
                    for cg in range(2):
                        nc.tensor.matmul(
                            psts[2 * qt + cg][:],
                            gT[:, gc, 128 * qt:128 * (qt + 1)],
                            wt[:, 512 * cg:512 * (cg + 1)],
                            start=(gc == 0), stop=False)
            for qt in range(4):
                ot = osb.tile([128, D], F32, tag="ot", name=f"ot_{qt}")
                for cg in range(2):
                    nc.tensor.matmul(psts[2 * qt + cg][:], ones_r[:],
                                     b2r_sb[:, 512 * cg:512 * (cg + 1)],
                                     start=False, stop=True)
                    nc.vector.tensor_add(ot[:, 512 * cg:512 * (cg + 1)],
                                         psts[2 * qt + cg][:],
                                         h_sb[:, qt, 512 * cg:512 * (cg + 1)])
                nc.sync.dma_start(io["out"][128 * qt:128 * (qt + 1), :], ot[:])

        if "xdump" in io:
            nc.sync.dma_start(io["xdump"][:], xln_d[:])
            nc.sync.dma_start(io["odump"][:, :, 0:256], o_sendA[:])
            nc.sync.dma_start(io["odump"][:, :, 256:512], o_sendB[:])
            nc.sync.dma_start(io["hdump"][:], h2_d[:])


# ---------------------------------------------------------------------------
# Host side
# ---------------------------------------------------------------------------

_CACHE = {}
LAST_RESULT = None  # BassKernelResults of the most recent run (for test.py)


def _get_program(key):
    if key not in _CACHE:
        _CACHE[key] = build_program(*key)
    return _CACHE[key]


def prepare_inputs(hidden_states, Wq, bq, Wk, bk, Wv, bv, Wp, bp,
                   ln1_g, ln1_b, ln2_g, ln2_b, W1, b1, W2, b2):
    import ml_dtypes
    bf = lambda a: np.ascontiguousarray(a.astype(ml_dtypes.bfloat16))
    chunk_major = lambda v: np.ascontiguousarray(v.reshape(-1, 128).T)

    # static masks
    kp = np.arange(128)[:, None]
    iq = np.arange(256)[None, :]
    maskd = np.zeros((128, 2, 2, 256), dtype=np.float32)
    maskd[:, 0][(kp > iq)[:, None, :].repeat(2, 1)] = NEG        # tile 2jq
    maskd[:, 1][(kp + 128 > iq)[:, None, :].repeat(2, 1)] = NEG  # tile 2jq+1
    colz = np.ones((128, NT), dtype=np.float32)
    for t in range(NT):
        colz[((128 * t + np.arange(128)) % JD) == (JD - 1), t] = 0.0
    # same mask along the free (kpos) axis of vT, per 512-wide rowgroup
    kz = ((np.arange(S) % JD) != (JD - 1)).astype(np.float32)  # [2048]
    vmask = np.broadcast_to(kz.reshape(4, 512)[None, :, :],
                            (128, 4, 512)).astype(ml_dtypes.bfloat16)
    colz2 = np.repeat(colz[:, :, None], 2, axis=2).astype(ml_dtypes.bfloat16)

    hs_flat = np.ascontiguousarray(hidden_states.reshape(NROW, D))
    w1x = np.ascontiguousarray(
        W1.reshape(DC, 128, GC, 128).transpose(2, 1, 0, 3))
    shared = dict(hs=hs_flat, wp=bf(Wp), w1=bf(w1x), w2=bf(W2),
                  b1l=chunk_major(b1), bpr=bf(bp.reshape(1, D)),
                  b2r=bf(b2.reshape(1, D)),
                  onesr=np.ones((1, 128), dtype=ml_dtypes.bfloat16),
                  ln1gb=np.stack([ln1_g, ln1_b]),
                  ln2gb=np.stack([ln2_g, ln2_b]),
                  maskd=maskd, colz=colz,
                  trim=np.ascontiguousarray(
                      (maskd == 0.0).astype(ml_dtypes.bfloat16)),
                  vmask=np.ascontiguousarray(vmask),
                  colz2=np.ascontiguousarray(colz2),
                  ident=np.eye(128, dtype=ml_dtypes.bfloat16))

    in_maps = []
    for core in range(NCORE):
        sl = slice(128 * core, 128 * (core + 1))
        m = dict(shared)
        m["hso"] = np.ascontiguousarray(hs_flat[R * core:R * (core + 1), :])
        m["wq"] = bf(Wq[:, sl])
        m["wk"] = bf(Wk[:, sl])
        m["wv"] = bf(Wv[:, sl])
        m["bq8"] = np.ascontiguousarray((bq[sl] * 0.125).reshape(128, 1))
        m["bk1"] = np.ascontiguousarray(bk[sl].reshape(128, 1))
        m["bv1"] = np.ascontiguousarray(bv[sl].reshape(128, 1))
        in_maps.append(m)
    return in_maps


def kernel(hidden_states, Wq, bq, Wk, bk, Wv, bv, Wp, bp,
           ln1_g, ln1_b, ln2_g, ln2_b, W1, b1, W2, b2):
    f32 = lambda a: np.ascontiguousarray(np.asarray(a, dtype=np.float32))
    hidden_states = f32(hidden_states)
    Wq, bq, Wk, bk, Wv, bv, Wp, bp = map(f32, (Wq, bq, Wk, bk, Wv, bv, Wp, bp))
    ln1_g, ln1_b, ln2_g, ln2_b = map(f32, (ln1_g, ln1_b, ln2_g, ln2_b))
    W1, b1, W2, b2 = map(f32, (W1, b1, W2, b2))

    apply_ln1 = bool(np.any(ln1_g != 1.0) or np.any(ln1_b != 0.0))
    apply_ln2 = bool(np.any(ln2_g != 1.0) or np.any(ln2_b != 0.0))
    nc = _get_program((apply_ln1, apply_ln2))
    in_maps = prepare_inputs(hidden_states, Wq, bq, Wk, bk, Wv, bv, Wp, bp,
                             ln1_g, ln1_b, ln2_g, ln2_b, W1, b1, W2, b2)

    res = run_bass_kernel_spmd(nc, in_maps, core_ids=list(range(NCORE)))
    global LAST_RESULT
    LAST_RESULT = res

    out_full = np.empty((NROW, D), dtype=np.float32)
    for core in range(NCORE):
        out_full[R * core:R * (core + 1), :] = res.results[core]["out"]
    return out_full.reshape(B, S, D)
